# revision 1
# baseline (speedup 1.0000x reference)
"""Bilinear interpolation (dense warp) Trainium2 kernel.

Strategy: pure data-parallel over batch (8 images per NeuronCore x 8 cores).
Per core, each image is processed in 128-row bands.  Since the displacement
field is N(0,1) (|d| < 6), every sampled point lies within a +-6 pixel window
of its output location.  The gather is computed as an exact masked 13x13
window sum:

  out[r,c] = sum_sy Wy_sy[r,c] * sum_sx Wx_sx[r,c] * I[r+sy, c+sx]

  Wx_s = (x0c==c+s)*(x1f-x) + (x1c==c+s)*(x-x0f)   (exact, incl. borders:
  Wy_s analogous)                                   clip + trunc + collapse)

Engine APs cannot start at arbitrary partitions, so the row (partition)
shift r+sy is done on the TensorEngine with shifted-identity matmuls into
PSUM; the column shift is a free-dim AP offset.  Window products run on the
VectorEngine and are accumulated on the TensorEngine via identity matmuls.
"""
import sys

sys.path.insert(0, "/opt/trn_rl_repo")
from contextlib import ExitStack

import numpy as np

from concourse import bass, mybir
import concourse.tile as tile
from concourse.bass_utils import run_bass_kernel_spmd
from concourse.masks import make_identity
from concourse.vector_clock import ScopedClock
import bass_rust

# --- workaround: this walrus build rejects >2 sem waits on one instruction;
# TileContext's tail drain carries the whole global clock.  Redistribute.
def _patched_drain_and_barrier(self, tick_clock, wait_clock):
    drain_inst = self.nc.sync.drain()
    wait_clock.add_sem_waits(
        drain_inst.ins, ScopedClock({None: tick_clock.global_clock})
    )
    si = drain_inst.ins.sync_info
    if si is not None and si.on_wait and len(si.on_wait) > 1:
        waits = list(si.on_wait)
        si.on_wait = [waits[0]]
        sems = {h.name: h for h in self.sems.allocated().values()}
        for w in waits[1:]:
            h = sems.get(w.ant_name)
            assert h is not None, (w.ant_name, list(sems))
            assert w.wait_mode == "sem-ge-imm", w
            self.nc.sync.wait_ge(h, w.wait_value)
    self.nc.all_engine_barrier()
    assert self.sems is not None
    popped = self.nc._tile_sem_poison_stack.pop()
    assert popped is self._sem_poison
    self.nc.clear_and_free_semaphores(list(self.sems.allocated().values()))
    self.nc.all_engine_barrier()


tile.TileContext._drain_and_barrier = _patched_drain_and_barrier

# --- same walrus limit, general case: split any scheduled instruction that
# carries >1 sem wait into single-wait NoOps ahead of it (same engine, same
# position in the engine stream -> semantically identical).
_MAXW = 1
_nop_counter = [0]


def _split_multiwaits(ordered):
    for bb_name, insts in ordered.items():
        out = []
        changed = False
        for inst in insts:
            si = getattr(inst, "sync_info", None)
            if si is not None and si.on_wait and len(si.on_wait) > _MAXW:
                waits = list(si.on_wait)
                for w in waits[:-_MAXW]:
                    _nop_counter[0] += 1
                    nop = mybir.InstNoOp(
                        name=f"I-wsplit-{_nop_counter[0]}", ins=[], outs=[]
                    )
                    nop.engine = inst.engine
                    nop.sync_info = mybir.SyncInfo(on_wait=[w], on_update=[])
                    out.append(nop)
                si.on_wait = waits[-_MAXW:]
                changed = True
            out.append(inst)
        if changed:
            insts[:] = out


_orig_lower_ordered = tile.TileContext._lower_ordered_insts


def _patched_lower_ordered(self, ordered):
    _split_multiwaits(ordered)
    return _orig_lower_ordered(self, ordered)


tile.TileContext._lower_ordered_insts = _patched_lower_ordered

H = W = 512
IPC = 8  # images per core
NCORES = 8
PAD = 6
WPAD = W + 2 * PAD  # 524
NS = 13  # window positions; s=0..12 <-> shift s-6
F32 = mybir.dt.float32
I32 = mybir.dt.int32
AL = mybir.AluOpType

TILES = [(0, 116), (116, 116), (232, 116), (348, 116), (464, 48)]


def _do_tile(nc, pools, consts, img, r0, nr, dram):
    imgs_d, dvx_d, dvy_d, out_d = dram
    iota_c, ident, shifts = consts
    pl_band, pl_dv, pl_scr, pl_w, pl_prod, pl_io, pl_psum, pl_psib = pools

    b0 = r0 - PAD
    lo, hi = max(0, b0), min(H, b0 + 128)
    nband = hi - lo
    dshift = r0 - lo  # band partition k holds image row lo+k

    IB = pl_band.tile([128, WPAD], F32, tag="band", name="band")
    if nband < 128:
        nc.vector.memset(IB[:], 0.0)
    nc.sync.dma_start(out=IB[:nband, :], in_=imgs_d[img, lo:hi, :])

    DVX = pl_dv.tile([128, W], F32, tag="dvx", name="dvx")
    nc.sync.dma_start(out=DVX[:nr], in_=dvx_d[img, r0 : r0 + nr, :])
    DVY = pl_dv.tile([128, W], F32, tag="dvy", name="dvy")
    nc.sync.dma_start(out=DVY[:nr], in_=dvy_d[img, r0 : r0 + nr, :])

    def t(tag, dtype=F32):
        return pl_scr.tile([128, W], dtype, tag=tag, name=tag)

    # per-partition row constants for this tile
    rbi = pl_scr.tile([128, 1], I32, tag="rbi", name="rbi")
    nc.gpsimd.iota(rbi[:], pattern=[[0, 1]], base=r0, channel_multiplier=1)
    rbY = pl_scr.tile([128, 1], F32, tag="rbY", name="rbY")
    nc.vector.tensor_copy(out=rbY[:], in_=rbi[:])
    rb6 = pl_scr.tile([128, 1], F32, tag="rb6", name="rb6")  # 6 - (r0 + p)
    nc.vector.tensor_scalar(
        out=rb6[:], in0=rbY[:], scalar1=-1.0, scalar2=6.0, op0=AL.mult, op1=AL.add
    )

    X = t("X")
    nc.vector.tensor_tensor(out=X[:nr], in0=DVX[:nr], in1=iota_c[:nr], op=AL.add)
    Y = t("Y")
    nc.vector.tensor_scalar(
        out=Y[:nr], in0=DVY[:nr], scalar1=rbY[:nr], scalar2=None, op0=AL.add
    )
    # The HW f32->i32 cast rounds to nearest; build floor and the reference's
    # trunc-toward-zero explicitly.  After clipping, trunc == floor for the
    # low corner; only the +1 corner needs the toward-zero adjustment
    # adj = (floor<0)&(floor!=v).
    def floor_planes(V, pfx):
        ci = t("fci", I32)
        nc.vector.tensor_copy(out=ci[:nr], in_=V[:nr])  # round-to-nearest
        cf = t("fcf")
        nc.vector.tensor_copy(out=cf[:nr], in_=ci[:nr])
        gt = t("fgt")
        nc.vector.tensor_tensor(out=gt[:nr], in0=cf[:nr], in1=V[:nr], op=AL.is_gt)
        fl = t("ffl")
        nc.vector.tensor_sub(out=fl[:nr], in0=cf[:nr], in1=gt[:nr])
        ne = t("fne")
        nc.vector.tensor_tensor(out=ne[:nr], in0=fl[:nr], in1=V[:nr], op=AL.not_equal)
        adj = t("fadj")  # (fl < 0) * (fl != v)
        nc.vector.scalar_tensor_tensor(
            out=adj[:nr], in0=fl[:nr], scalar=0.0, in1=ne[:nr],
            op0=AL.is_lt, op1=AL.mult,
        )
        lo = t(pfx + "lo")  # clip(floor, 0, 511)
        nc.vector.tensor_scalar(
            out=lo[:nr], in0=fl[:nr], scalar1=0.0, scalar2=511.0,
            op0=AL.max, op1=AL.min,
        )
        hi = t(pfx + "hi")  # clip(trunc + 1, 0, 511);  trunc+1 = fl + adj + 1
        nc.vector.scalar_tensor_tensor(
            out=hi[:nr], in0=adj[:nr], scalar=1.0, in1=fl[:nr],
            op0=AL.add, op1=AL.add,
        )
        nc.vector.tensor_scalar(
            out=hi[:nr], in0=hi[:nr], scalar1=0.0, scalar2=511.0,
            op0=AL.max, op1=AL.min,
        )
        return lo, hi

    X0, X1 = floor_planes(X, "x")
    Y0, Y1 = floor_planes(Y, "y")

    WXA = t("WXA")
    nc.vector.tensor_sub(out=WXA[:nr], in0=X1[:nr], in1=X[:nr])
    WXB = t("WXB")
    nc.vector.tensor_sub(out=WXB[:nr], in0=X[:nr], in1=X0[:nr])
    WYA = t("WYA")
    nc.vector.tensor_sub(out=WYA[:nr], in0=Y1[:nr], in1=Y[:nr])
    WYB = t("WYB")
    nc.vector.tensor_sub(out=WYB[:nr], in0=Y[:nr], in1=Y0[:nr])

    # window index planes (exact small integers in f32)
    JX0 = t("JX0")
    nc.vector.scalar_tensor_tensor(
        out=JX0[:nr], in0=X0[:nr], scalar=6.0, in1=iota_c[:nr],
        op0=AL.add, op1=AL.subtract,
    )
    JX1 = t("JX1")
    nc.vector.scalar_tensor_tensor(
        out=JX1[:nr], in0=X1[:nr], scalar=6.0, in1=iota_c[:nr],
        op0=AL.add, op1=AL.subtract,
    )
    JY0 = t("JY0")
    nc.vector.tensor_scalar(
        out=JY0[:nr], in0=Y0[:nr], scalar1=rb6[:nr], scalar2=None, op0=AL.add
    )
    JY1 = t("JY1")
    nc.vector.tensor_scalar(
        out=JY1[:nr], in0=Y1[:nr], scalar1=rb6[:nr], scalar2=None, op0=AL.add
    )

    # weight planes; the 13 x-planes live in ONE wide tile so the window
    # products can run as a single wide instruction per sy
    WXall = pl_w.tile([128, NS * W], F32, tag="wxall", name="wxall")

    def weight_plane(s, j0, j1, wa, wb, out_ap):
        t1 = t("wt1")
        nc.vector.scalar_tensor_tensor(
            out=t1[:nr], in0=j0[:nr], scalar=float(s), in1=wa[:nr],
            op0=AL.is_equal, op1=AL.mult,
        )
        t2 = t("wt2")
        nc.vector.scalar_tensor_tensor(
            out=t2[:nr], in0=j1[:nr], scalar=float(s), in1=wb[:nr],
            op0=AL.is_equal, op1=AL.mult,
        )
        nc.vector.tensor_add(out=out_ap, in0=t1[:nr], in1=t2[:nr])

    for s in range(NS):
        weight_plane(s, JX0, JX1, WXA, WXB, WXall[:nr, s * W : (s + 1) * W])
    WYs = []
    for s in range(NS):
        w = pl_w.tile([128, W], F32, tag=f"wy{s}", name=f"wy{s}")
        weight_plane(s, JY0, JY1, WYA, WYB, w[:nr])
        WYs.append(w)

    VP = pl_psum.tile([128, W], F32, tag="V", name="V")
    OP = pl_psum.tile([128, W], F32, tag="O", name="O")
    for isy in range(NS):
        sy = isy - 6
        delta = sy + dshift  # band partition of row r0+p+sy is p+delta
        M = shifts[delta + 6]  # [k, p] one iff k == p + delta
        # shift band rows into output alignment via PE (psum <- M.T @ IB)
        SIBp = pl_psib.tile([128, WPAD], F32, tag="SIBp", name="SIBp")
        nc.tensor.matmul(
            SIBp[:nr, 0:W], lhsT=M[:, :nr], rhs=IB[:, 0:W], start=True, stop=True,
        )
        nc.tensor.matmul(
            SIBp[:nr, W:WPAD], lhsT=M[:, :nr], rhs=IB[:, W:WPAD],
            start=True, stop=True,
        )
        SIB = pl_prod.tile([128, WPAD], F32, tag="SIB", name="SIB", bufs=2)
        nc.scalar.copy(out=SIB[:nr], in_=SIBp[:nr])
        # all 13 window products in one wide instruction: in1 reads the
        # overlapping windows SIB[p, sx + c] via a [.., NS, W] strided AP
        prod = pl_prod.tile([128, NS * W], F32, tag="prod", name="prod", bufs=2)
        sibwin = SIB[:nr].copy()
        sibwin.ap = bass_rust.VecI64Pair(
            [list(sibwin.ap[0]), [1, NS], [1, W]]
        )
        nc.vector.tensor_mul(
            out=prod[:nr].rearrange("p (a c) -> p a c", a=NS),
            in0=WXall[:nr].rearrange("p (a c) -> p a c", a=NS),
            in1=sibwin,
        )
        for isx in range(NS):
            nc.tensor.matmul(
                VP[:nr], lhsT=ident[:nr, :nr], rhs=prod[:nr, isx * W : (isx + 1) * W],
                start=(isx == 0), stop=(isx == NS - 1), skip_group_check=True,
            )
        yp = pl_prod.tile([128, W], F32, tag="yp", name="yp", bufs=2)
        nc.vector.tensor_mul(out=yp[:nr], in0=VP[:nr], in1=WYs[isy][:nr])
        nc.tensor.matmul(
            OP[:nr], lhsT=ident[:nr, :nr], rhs=yp[:nr],
            start=(isy == 0), stop=(isy == NS - 1), skip_group_check=True,
        )
    outs = pl_io.tile([128, W], F32, tag="outs", name="outs")
    nc.scalar.copy(out=outs[:nr], in_=OP[:nr])
    nc.sync.dma_start(out=out_d[img, r0 : r0 + nr, :], in_=outs[:nr])


def _build():
    nc = bass.Bass()
    imgs_d = nc.dram_tensor("imgs", [IPC, H, WPAD], F32, kind="ExternalInput").ap()
    dvx_d = nc.dram_tensor("dvx", [IPC, H, W], F32, kind="ExternalInput").ap()
    dvy_d = nc.dram_tensor("dvy", [IPC, H, W], F32, kind="ExternalInput").ap()
    out_d = nc.dram_tensor("out", [IPC, H, W], F32, kind="ExternalOutput").ap()
    dram = (imgs_d, dvx_d, dvy_d, out_d)

    with ExitStack() as ctx:
        tc = ctx.enter_context(tile.TileContext(nc))
        pl_const = ctx.enter_context(tc.tile_pool(name="const", bufs=1))
        pl_band = ctx.enter_context(tc.tile_pool(name="band", bufs=2))
        pl_dv = ctx.enter_context(tc.tile_pool(name="dv", bufs=2))
        pl_scr = ctx.enter_context(tc.tile_pool(name="scr", bufs=1))
        pl_w = ctx.enter_context(tc.tile_pool(name="w", bufs=1))
        pl_prod = ctx.enter_context(tc.tile_pool(name="prod", bufs=2))
        pl_io = ctx.enter_context(tc.tile_pool(name="io", bufs=2))
        pl_psum = ctx.enter_context(tc.tile_pool(name="psum", bufs=2, space="PSUM"))
        pl_psib = ctx.enter_context(tc.tile_pool(name="psib", bufs=2, space="PSUM"))

        iota_i = pl_const.tile([128, W], I32, name="iota_i")
        nc.gpsimd.iota(iota_i[:], pattern=[[1, W]], base=0, channel_multiplier=0)
        iota_c = pl_const.tile([128, W], F32, name="iota_c")
        nc.vector.tensor_copy(out=iota_c[:], in_=iota_i[:])
        ident = pl_const.tile([128, 128], F32, name="ident")
        make_identity(nc, ident[:])

        # shifted identities M_delta[k, j] = 1 iff k == j + delta, delta in [-6, 12]
        pcol_i = pl_const.tile([128, 1], I32, name="pcol_i")
        nc.gpsimd.iota(pcol_i[:], pattern=[[0, 1]], base=0, channel_multiplier=1)
        pcol = pl_const.tile([128, 1], F32, name="pcol")
        nc.vector.tensor_copy(out=pcol[:], in_=pcol_i[:])
        diag2 = pl_const.tile([128, 128], F32, name="diag2")  # j - k
        nc.vector.tensor_scalar(
            out=diag2[:], in0=iota_c[:, :128], scalar1=pcol[:], scalar2=None,
            op0=AL.subtract,
        )
        shifts = []
        for delta in range(-6, 13):
            m = pl_const.tile([128, 128], F32, name=f"shift{delta + 6}")
            nc.vector.tensor_scalar(
                out=m[:], in0=diag2[:], scalar1=float(-delta), scalar2=None,
                op0=AL.is_equal,
            )
            shifts.append(m)

        pools = (pl_band, pl_dv, pl_scr, pl_w, pl_prod, pl_io, pl_psum, pl_psib)
        consts = (iota_c, ident, shifts)
        for img in range(IPC):
            for r0, nr in TILES:
                _do_tile(nc, pools, consts, img, r0, nr, dram)
    return nc


_nc_cache = None


def kernel(imgs: np.ndarray, dvfs: np.ndarray) -> np.ndarray:
    global _nc_cache
    B = imgs.shape[0]
    assert imgs.shape == (B, H, W, 1) and dvfs.shape == (B, H, W, 2)
    per = B // NCORES

    imgs_p = np.zeros((B, H, WPAD), np.float32)
    imgs_p[:, :, PAD : PAD + W] = imgs[..., 0]
    dvx = np.ascontiguousarray(dvfs[..., 0])
    dvy = np.ascontiguousarray(dvfs[..., 1])

    if _nc_cache is None:
        _nc_cache = _build()
    nc = _nc_cache

    in_maps = [
        {
            "imgs": imgs_p[i * per : (i + 1) * per],
            "dvx": dvx[i * per : (i + 1) * per],
            "dvy": dvy[i * per : (i + 1) * per],
        }
        for i in range(NCORES)
    ]
    res = run_bass_kernel_spmd(nc, in_maps, list(range(NCORES)))
    out = np.concatenate([res.results[i]["out"] for i in range(NCORES)], axis=0)
    return out[..., None].astype(np.float32)



# revision 11
# speedup vs baseline: 1.1301x; 1.1301x over previous
"""Bilinear interpolation (dense warp) Trainium2 kernel, v2.

Data-parallel over batch (8 images/core x 8 cores).  Per 128-row tile the
sampled value is a separable 13-tap tent-weighted window sum

  out[r,c] = sum_m tenty_m(y) * Z_m[r,c]
  Z_m[r,c] = sum_k tentx_k(x) * I[r+m-6, c+k-6]     (tent = relu(1-|d|))

computed entirely in fp16 on the DVE (products, 2x mode) with PE
identity-matmul accumulation in PSUM.  The 13 row-shifted bands are DMA'd
straight from DRAM (edge rows/cols replicate-padded).  Reference border
semantics (trunc-toward-zero + clip with weights from clipped corners)
reduce to: linear extrapolation for x|y in (-1,0) -- folded into the tent
planes via diagonal-AP fixups -- and exact zero for x|y <= -1 or >= 511,
applied as a final threshold mask.
"""
import sys

sys.path.insert(0, "/opt/trn_rl_repo")
from contextlib import ExitStack

import numpy as np

from concourse import bass, mybir
import concourse.tile as tile
from concourse.bass_utils import run_bass_kernel_spmd
from concourse.vector_clock import ScopedClock
import bass_rust

# --- workaround: this walrus build rejects >2 sem waits on one instruction;
# TileContext's tail drain carries the whole global clock.  Redistribute.
def _patched_drain_and_barrier(self, tick_clock, wait_clock):
    drain_inst = self.nc.sync.drain()
    wait_clock.add_sem_waits(
        drain_inst.ins, ScopedClock({None: tick_clock.global_clock})
    )
    si = drain_inst.ins.sync_info
    if si is not None and si.on_wait and len(si.on_wait) > 1:
        waits = list(si.on_wait)
        si.on_wait = [waits[0]]
        sems = {h.name: h for h in self.sems.allocated().values()}
        for w in waits[1:]:
            h = sems.get(w.ant_name)
            assert h is not None, (w.ant_name, list(sems))
            assert w.wait_mode == "sem-ge-imm", w
            self.nc.sync.wait_ge(h, w.wait_value)
    self.nc.all_engine_barrier()
    assert self.sems is not None
    popped = self.nc._tile_sem_poison_stack.pop()
    assert popped is self._sem_poison
    self.nc.clear_and_free_semaphores(list(self.sems.allocated().values()))
    self.nc.all_engine_barrier()


tile.TileContext._drain_and_barrier = _patched_drain_and_barrier

# --- same walrus limit, general case: split any scheduled instruction that
# carries >1 sem wait into single-wait NoOps ahead of it.
_MAXW = 1
_nop_counter = [0]


def _split_multiwaits(ordered):
    for bb_name, insts in ordered.items():
        out = []
        changed = False
        for inst in insts:
            si = getattr(inst, "sync_info", None)
            if si is not None and si.on_wait and len(si.on_wait) > _MAXW:
                waits = list(si.on_wait)
                for w in waits[:-_MAXW]:
                    _nop_counter[0] += 1
                    nop = mybir.InstNoOp(
                        name=f"I-wsplit-{_nop_counter[0]}", ins=[], outs=[]
                    )
                    nop.engine = inst.engine
                    nop.sync_info = mybir.SyncInfo(on_wait=[w], on_update=[])
                    out.append(nop)
                si.on_wait = waits[-_MAXW:]
                changed = True
            out.append(inst)
        if changed:
            insts[:] = out


_orig_lower_ordered = tile.TileContext._lower_ordered_insts


def _patched_lower_ordered(self, ordered):
    _split_multiwaits(ordered)
    return _orig_lower_ordered(self, ordered)


tile.TileContext._lower_ordered_insts = _patched_lower_ordered

H = W = 512
IPC = 8  # images per core
NCORES = 8
PAD = 6
WPAD = W + 2 * PAD  # 524
NS = 13  # window taps (shift m-6, m = 0..12)
F16 = mybir.dt.float16
F32 = mybir.dt.float32
AL = mybir.AluOpType
AF = mybir.ActivationFunctionType

# const layout (element offsets within the [128, NCONST] f16 const tile)
O_THHI = 0          # [*, 512]  511 - c
O_THLO = 512        # [*, 512]  -1 - c
O_RM2 = 1024        # [*, 13*512]  y-extrap mask (+1 @ m=7-p, -1 @ m=6-p, p<6)
O_ID = 1024 + NS * W          # [*, 128] +identity
O_NEGID = O_ID + 128          # [*, 128] -identity
NCONST = O_NEGID + 128
# f32 per-partition consts (cstf): [:,t]=511-(r0_t+p); [:,4+t]=-1-(r0_t+p);
# [:,8+m]=6-m (abs bias); [:,21]=1.0 (relu bias)


def _ap(base_ap, dims, elem_offset):
    """Clone base_ap with custom free dims (strides/offset in elements)."""
    a = base_ap.copy()
    part = list(a.ap[0])
    a.ap = bass_rust.VecI64Pair([part] + [list(d) for d in dims])
    if elem_offset:
        a.offset = a.offset + elem_offset
    return a


def _do_tile(nc, pools, consts, img, t, dram):
    imgs_d, dvx_d, dvy_d, out_d = dram
    CST, CSTF = consts
    (pl_dv, pl_sib, pl_scd, pl_tx, pl_ty, pl_prod, pl_zsb, pl_sc, pl_io,
     pl_psz, pl_pso) = pools
    r0 = 128 * t

    DVX = pl_dv.tile([128, W], F16, tag="dvx", name="dvx")
    nc.sync.dma_start(out=DVX[:], in_=dvx_d[img, r0 : r0 + 128])
    DVY = pl_dv.tile([128, W], F16, tag="dvy", name="dvy")
    nc.sync.dma_start(out=DVY[:], in_=dvy_d[img, r0 : r0 + 128])

    SIBs = []
    for m in range(NS):
        sib = pl_sib.tile([128, WPAD], F16, tag=f"sib{m}", name=f"sib{m}")
        base = r0 + m - 6  # image row of partition 0
        p0 = max(0, -base)
        p1 = min(128, H - base)
        nc.sync.dma_start(
            out=sib[p0:p1, :], in_=imgs_d[img, base + p0 : base + p1, :]
        )
        for p in range(0, p0):  # replicate row 0 (tile 0 edge)
            nc.sync.dma_start(out=sib[p : p + 1, :], in_=imgs_d[img, 0:1, :])
        for p in range(p1, 128):  # replicate row 511 (tile 3 edge)
            nc.sync.dma_start(
                out=sib[p : p + 1, :], in_=imgs_d[img, H - 1 : H, :]
            )
        SIBs.append(sib)

    # ---- masks: zero where x or y outside (-1, 511) ----------------------
    border = t in (0, 3)
    thhi = _ap(CST[:], [[1, W]], O_THHI)
    thlo = _ap(CST[:], [[1, W]], O_THLO)
    M = pl_sc.tile([128, W], F16, tag="mask", name="mask")
    M2 = pl_sc.tile([128, W], F16, tag="mask2", name="mask2")
    nc.vector.tensor_tensor(out=M[:], in0=DVX[:], in1=thhi, op=AL.is_lt)
    nc.vector.tensor_tensor(out=M2[:], in0=DVX[:], in1=thlo, op=AL.is_gt)
    nc.vector.tensor_tensor(out=M[:], in0=M[:], in1=M2[:], op=AL.mult)
    if border:
        thyhi = CSTF[:, t : t + 1]
        thylo = CSTF[:, 4 + t : 4 + t + 1]
        MY = pl_sc.tile([128, W], F16, tag="masky", name="masky")
        nc.vector.tensor_scalar(
            out=MY[:], in0=DVY[:], scalar1=thyhi, scalar2=None, op0=AL.is_lt
        )
        nc.vector.tensor_tensor(out=M[:], in0=M[:], in1=MY[:], op=AL.mult)
        nc.vector.tensor_scalar(
            out=MY[:], in0=DVY[:], scalar1=thylo, scalar2=None, op0=AL.is_gt
        )
        nc.vector.tensor_tensor(out=M[:], in0=M[:], in1=MY[:], op=AL.mult)

    # ---- X tent planes (negated: TXk = min(|dx-(k-6)|-1, 0)), DVE -------
    TX = pl_tx.tile([128, NS * W], F16, tag="tx", name="tx")
    for k in range(NS):
        T1 = pl_sc.tile([128, W], F16, tag="t1", name=f"t1_{k}")
        nc.scalar.activation(
            out=T1[:], in_=DVX[:], func=AF.Abs,
            bias=CSTF[:, 8 + k : 9 + k], scale=1.0,
        )
        nc.vector.tensor_scalar(
            out=TX[:, k * W : (k + 1) * W], in0=T1[:], scalar1=1.0,
            scalar2=0.0, op0=AL.subtract, op1=AL.min,
        )
    # x-extrap fix for x in (-1,0) at cols 0..5: tent_{7-c} += minX,
    # tent_{6-c} -= minX  (negated TX: -=, += respectively)
    X6 = pl_sc.tile([128, 8], F16, tag="x6", name="x6")
    # X6[:, c] = dx + c  (c = 0..5): dvx col c plus c
    for c in range(6):
        nc.vector.tensor_scalar(
            out=X6[:, c : c + 1], in0=DVX[:, c : c + 1], scalar1=float(c),
            scalar2=0.0, op0=AL.add, op1=AL.min,
        )
    d7 = _ap(TX[:], [[-(W - 1), 6]], 7 * W)
    d6 = _ap(TX[:], [[-(W - 1), 6]], 6 * W)
    x6 = X6[:, 0:6]
    nc.vector.tensor_tensor(out=d7, in0=d7, in1=x6, op=AL.subtract)
    nc.vector.tensor_tensor(out=d6, in0=d6, in1=x6, op=AL.add)

    # ---- Y tent planes (positive: relu(1-|dy-(m-6)|)), ScalarE ----------
    TY = pl_ty.tile([128, NS * W], F16, tag="ty", name="ty")
    for m in range(NS):
        nc.scalar.activation(
            out=TY[:, m * W : (m + 1) * W], in_=DVY[:], func=AF.Abs,
            bias=CSTF[:, 8 + m : 9 + m], scale=1.0,
        )
    for m in range(NS):
        sl = TY[:, m * W : (m + 1) * W]
        nc.scalar.activation(
            out=sl, in_=sl, func=AF.Relu, bias=CSTF[:, 21:22], scale=-1.0
        )
    if t == 0:
        # y-extrap for y in (-1,0) at rows 0..5: via RM2 const mask
        Yf = pl_sc.tile([128, W], F16, tag="yf", name="yf")
        prow = CSTF[:, 4:5]  # -1-p
        # minY = min(p + dy, 0)
        nc.vector.tensor_scalar(
            out=Yf[:], in0=DVY[:], scalar1=prow, scalar2=None, op0=AL.subtract
        )  # dy - (-1-p) = y + 1
        nc.vector.tensor_scalar(
            out=Yf[:], in0=Yf[:], scalar1=-1.0, scalar2=0.0,
            op0=AL.add, op1=AL.min,
        )  # min(y, 0)
        rm2 = _ap(CST[:], [[W, NS], [1, W]], O_RM2)
        ybc = _ap(Yf[:], [[0, NS], [1, W]], 0)
        FIX = pl_prod.tile([128, NS * W], F16, tag="prod", name="yfix")
        fx = _ap(FIX[:], [[W, NS], [1, W]], 0)
        nc.vector.tensor_tensor(out=fx, in0=rm2, in1=ybc, op=AL.mult)
        nc.vector.tensor_tensor(out=TY[:], in0=TY[:], in1=FIX[:], op=AL.add)

    ident = _ap(CST[:], [[1, 128]], O_ID)
    negid = _ap(CST[:], [[1, 128]], O_NEGID)

    # ---- per-shift x-stage + y accumulation -----------------------------
    OUTP = pl_pso.tile([128, W], F32, tag="outp", name="outp")
    for m in range(NS):
        sib = SIBs[m]
        scd = pl_scd.tile([128, WPAD - 1], F16, tag="scd", name="scd")
        nc.vector.tensor_copy(out=scd[:], in_=sib[:, 1:WPAD])
        prod = pl_prod.tile([128, NS * W], F16, tag="prod", name=f"prod{m}")
        # even taps k=0,2,..,12 -> prod planes 0..6
        pe_out = _ap(prod[:], [[W, 7], [1, W]], 0)
        pe_tx = _ap(TX[:], [[2 * W, 7], [1, W]], 0)
        pe_sib = _ap(sib[:], [[2, 7], [1, W]], 0)
        nc.vector.tensor_tensor(out=pe_out, in0=pe_tx, in1=pe_sib, op=AL.mult)
        # odd taps k=1,3,..,11 -> prod planes 7..12 (shifted copy, aligned)
        po_out = _ap(prod[:], [[W, 6], [1, W]], 7 * W)
        po_tx = _ap(TX[:], [[2 * W, 6], [1, W]], W)
        po_scd = _ap(scd[:], [[2, 6], [1, W]], 0)
        nc.vector.tensor_tensor(out=po_out, in0=po_tx, in1=po_scd, op=AL.mult)

        ZP = pl_psz.tile([128, W], F32, tag="z", name=f"z{m}")
        for i in range(NS):
            nc.tensor.matmul(
                ZP[:], lhsT=negid, rhs=prod[:, i * W : (i + 1) * W],
                start=(i == 0), stop=(i == NS - 1), skip_group_check=True,
            )
        zsb = pl_zsb.tile([128, W], F16, tag=f"zsb{m}", name=f"zsb{m}")
        nc.scalar.copy(out=zsb[:], in_=ZP[:])
        pry = pl_zsb.tile([128, W], F16, tag=f"pry{m}", name=f"pry{m}")
        nc.vector.tensor_tensor(
            out=pry[:], in0=TY[:, m * W : (m + 1) * W], in1=zsb[:], op=AL.mult
        )
        nc.tensor.matmul(
            OUTP[:], lhsT=ident, rhs=pry[:],
            start=(m == 0), stop=(m == NS - 1), skip_group_check=True,
        )

    OUTS = pl_io.tile([128, W], F32, tag="outs", name="outs")
    nc.vector.tensor_tensor(out=OUTS[:], in0=OUTP[:], in1=M[:], op=AL.mult)
    nc.sync.dma_start(out=out_d[img, r0 : r0 + 128, :], in_=OUTS[:])


def _host_consts():
    cst = np.zeros((128, NCONST), np.float16)
    c = np.arange(W, dtype=np.float32)
    p = np.arange(128, dtype=np.float32)
    cst[:, O_THHI : O_THHI + W] = (511.0 - c)[None, :]
    cst[:, O_THLO : O_THLO + W] = (-1.0 - c)[None, :]
    rm2 = np.zeros((128, NS, W), np.float32)
    for pp in range(6):
        rm2[pp, 7 - pp, :] = 1.0
        rm2[pp, 6 - pp, :] = -1.0
    cst[:, O_RM2 : O_RM2 + NS * W] = rm2.reshape(128, NS * W)
    ident = np.eye(128, dtype=np.float32)
    cst[:, O_ID : O_ID + 128] = ident
    cst[:, O_NEGID : O_NEGID + 128] = -ident
    cstf = np.zeros((128, 24), np.float32)
    for t in range(4):
        cstf[:, t] = 511.0 - (128 * t + p)
        cstf[:, 4 + t] = -1.0 - (128 * t + p)
    for m in range(NS):
        cstf[:, 8 + m] = 6.0 - m
    cstf[:, 21] = 1.0
    return cst, cstf


def _build():
    nc = bass.Bass()
    imgs_d = nc.dram_tensor("imgs", [IPC, H, WPAD], F16, kind="ExternalInput").ap()
    dvx_d = nc.dram_tensor("dvx", [IPC, H, W], F16, kind="ExternalInput").ap()
    dvy_d = nc.dram_tensor("dvy", [IPC, H, W], F16, kind="ExternalInput").ap()
    cst_d = nc.dram_tensor("cst", [128, NCONST], F16, kind="ExternalInput").ap()
    cstf_d = nc.dram_tensor("cstf", [128, 24], F32, kind="ExternalInput").ap()
    out_d = nc.dram_tensor("out", [IPC, H, W], F32, kind="ExternalOutput").ap()
    dram = (imgs_d, dvx_d, dvy_d, out_d)

    with ExitStack() as ctx:
        tc = ctx.enter_context(tile.TileContext(nc))
        pl_const = ctx.enter_context(tc.tile_pool(name="const", bufs=1))
        pl_dv = ctx.enter_context(tc.tile_pool(name="dv", bufs=2))
        pl_sib = ctx.enter_context(tc.tile_pool(name="sib", bufs=2))
        pl_scd = ctx.enter_context(tc.tile_pool(name="scd", bufs=2))
        pl_tx = ctx.enter_context(tc.tile_pool(name="tx", bufs=2))
        pl_ty = ctx.enter_context(tc.tile_pool(name="ty", bufs=2))
        pl_prod = ctx.enter_context(tc.tile_pool(name="prod", bufs=2))
        pl_zsb = ctx.enter_context(tc.tile_pool(name="zsb", bufs=2))
        pl_sc = ctx.enter_context(tc.tile_pool(name="sc", bufs=2))
        pl_io = ctx.enter_context(tc.tile_pool(name="io", bufs=2))
        pl_psz = ctx.enter_context(tc.tile_pool(name="psz", bufs=3, space="PSUM"))
        pl_pso = ctx.enter_context(tc.tile_pool(name="pso", bufs=2, space="PSUM"))

        CST = pl_const.tile([128, NCONST], F16, name="cst")
        nc.sync.dma_start(out=CST[:], in_=cst_d[:, :])
        CSTF = pl_const.tile([128, 24], F32, name="cstf")
        nc.sync.dma_start(out=CSTF[:], in_=cstf_d[:, :])

        pools = (pl_dv, pl_sib, pl_scd, pl_tx, pl_ty, pl_prod, pl_zsb, pl_sc,
                 pl_io, pl_psz, pl_pso)
        for img in range(IPC):
            for t in range(4):
                _do_tile(nc, pools, (CST, CSTF), img, t, dram)
    return nc


_nc_cache = None


def kernel(imgs: np.ndarray, dvfs: np.ndarray) -> np.ndarray:
    global _nc_cache
    B = imgs.shape[0]
    assert imgs.shape == (B, H, W, 1) and dvfs.shape == (B, H, W, 2)
    per = B // NCORES

    im = imgs[..., 0].astype(np.float16)
    imp = np.empty((B, H, WPAD), np.float16)
    imp[:, :, PAD : PAD + W] = im
    imp[:, :, :PAD] = im[:, :, :1]
    imp[:, :, PAD + W :] = im[:, :, -1:]
    # fp16 with round-toward-zero: the reference output is discontinuous at
    # the mask boundaries (x|y = -1 or 511, integer thresholds); truncation
    # keeps quantized coords on the same side of every boundary as the
    # original (nearest-rounding can land exactly on one and flip the side).
    def f16_trunc(a):
        b = np.ascontiguousarray(a, np.float32).view(np.uint32).copy()
        b &= np.uint32(0xFFFFE000)
        return b.view(np.float32).astype(np.float16)

    dvx = f16_trunc(dvfs[..., 0])
    dvy = f16_trunc(dvfs[..., 1])
    cst, cstf = _host_consts()

    if _nc_cache is None:
        _nc_cache = _build()
    nc = _nc_cache

    in_maps = [
        {
            "imgs": imp[i * per : (i + 1) * per],
            "dvx": dvx[i * per : (i + 1) * per],
            "dvy": dvy[i * per : (i + 1) * per],
            "cst": cst,
            "cstf": cstf,
        }
        for i in range(NCORES)
    ]
    res = run_bass_kernel_spmd(nc, in_maps, list(range(NCORES)))
    global LAST_RESULT
    LAST_RESULT = res
    out = np.concatenate([res.results[i]["out"] for i in range(NCORES)], axis=0)
    return out[..., None].astype(np.float32)


LAST_RESULT = None
